# revision 30
# baseline (speedup 1.0000x reference)
"""Multi-head attention (B=2, S=2048, E=768, H=8) on 8 Trainium2 NeuronCores.

Sharding: core i handles batch b = i//4 and heads {2*(i%4), 2*(i%4)+1}
(data parallel on B, tensor parallel on heads). Each core computes its
QKV projections (column-sliced weights), full attention for its 2 heads,
and a partial output projection (row-sliced Wo). The host sums the 4
partials per batch and adds the (adjusted) output bias.

Schedule (all PE work in fp16/bf16 at full rate, 2 cols/cycle):
- q/k biases are folded exactly into two augmented weight columns
  (head dim 96 -> 98), so energies = (xWq+bq)(xWk+bk)^T come out of a
  single matmul over fp16 q~/k~ tiles.
- Attention runs in 4 passes (2 heads x 2 query halves of 1024).  Each
  pass: per key tile kt, scores (2 matmuls) -> exp on the ACT engine ->
  PV (2 matmuls, accumulated in a [128,1024] PSUM tile).  The exp
  (1.1us per [128,1024]) paces the loop, so all remaining projection
  matmuls are interleaved as fillers to keep the PE dense.
- V' carries a built-in ones row (row 96 of vT), so the PV stationary
  slice [*, 0:97] directly yields the softmax row sums in partition 96
  with no copies or memsets of a separate V buffer.
- Normalization reads the PV PSUM directly: reciprocal of row 96,
  gpsimd partition-broadcast, one vector multiply -> bf16 attN.
- Output projection accumulates both heads per 128-query tile in PSUM,
  drains via vector cast to fp16, and streams fp16 tiles to HBM
  (halves the output DMA vs f32).  The host sums partials in f64.
- The V bias contributes a constant (softmax rows sum to 1), folded
  into bo on the host: bo' = bo + scaling * (bv @ Wo). The softmax
  "scaling" quirk is folded into Wo' = scaling * Wo.
"""

import numpy as np
import ml_dtypes

import concourse.mybir as mybir
import concourse.tile as tile
from concourse import bacc
from concourse import bass_utils

bf16 = ml_dtypes.bfloat16
F32 = mybir.dt.float32
BF = mybir.dt.bfloat16
F16 = mybir.dt.float16
AF = mybir.ActivationFunctionType

B, S, E, H, HD = 2, 2048, 768, 8, 96
HA = HD + 2          # augmented head dim (bias folding)
HPC = 2              # heads per core
N_CORES = 8
SCALING = HD ** -0.5
NE = E // 128        # 6 contraction tiles for projections
NT = S // 128        # 16 key tiles
QH = 2               # query halves of 1024
HQ = S // QH         # 1024

_CACHE = {}


def _build():
    nc = bacc.Bacc("TRN2", target_bir_lowering=False, debug=False,
                   enable_asserts=False, num_devices=N_CORES)

    xT = nc.dram_tensor("xT", [E, S], F16, kind="ExternalInput")
    wq = nc.dram_tensor("wq", [E, HPC * 128], F16, kind="ExternalInput")
    wkv = nc.dram_tensor("wkv", [E, 2 * HPC * 128], F16,
                         kind="ExternalInput")
    wo = nc.dram_tensor("wo", [HPC, HD, E], BF, kind="ExternalInput")
    bqa = nc.dram_tensor("bqa", [HA, HPC], F32, kind="ExternalInput")
    bka = nc.dram_tensor("bka", [HA, HPC], F32, kind="ExternalInput")
    out = nc.dram_tensor("out", [S, E], F16, kind="ExternalOutput")
    dbg = {}
    if _CACHE.get("debug"):
        dbg["qT00"] = nc.dram_tensor("d_qT00", [HA, HQ], F16,
                                     kind="ExternalOutput")
        dbg["kT00"] = nc.dram_tensor("d_kT00", [HA, HQ], F16,
                                     kind="ExternalOutput")
        dbg["kT01"] = nc.dram_tensor("d_kT01", [HA, HQ], F16,
                                     kind="ExternalOutput")
        dbg["vS0"] = nc.dram_tensor("d_vS0", [128, 16, 128], BF,
                                    kind="ExternalOutput")
        dbg["aN0"] = nc.dram_tensor("d_aN0", [HD, S], BF,
                                    kind="ExternalOutput")
        dbg["aN1"] = nc.dram_tensor("d_aN1", [HD, S], BF,
                                    kind="ExternalOutput")
        dbg["att00"] = nc.dram_tensor("d_att00", [128, HQ], F32,
                                      kind="ExternalOutput")
        dbg["pt0"] = nc.dram_tensor("d_pt0", [128, HQ], F32,
                                    kind="ExternalOutput")
        dbg["rb0"] = nc.dram_tensor("d_rb0", [HD, 512], F32,
                                    kind="ExternalOutput")

    with tile.TileContext(nc) as tc:
        with tc.tile_pool(name="pw", bufs=1) as pw, \
             tc.tile_pool(name="pvt", bufs=4) as pvt, \
             tc.tile_pool(name="ppr", bufs=18) as ppr, \
             tc.tile_pool(name="pno", bufs=2) as pno, \
             tc.tile_pool(name="pout", bufs=4) as pout, \
             tc.tile_pool(name="pa", bufs=1, space="PSUM") as pa, \
             tc.tile_pool(name="pb", bufs=3, space="PSUM") as pb:

            # ---------- input DMAs ----------
            # wq tiles stay small (fast first arrival); k/v weights merge
            # into one [128, 2, 256] tile per e-block.  x half tiles are on
            # the gpsimd issue queue, with the first one split across both
            # queues.  Each dma_start costs ~650ns of serial issue time.
            wq_t = [pw.tile([128, HPC * 128], F16, tag=f"wq{e}",
                            name=f"wq{e}") for e in range(NE)]
            kv_t = [pw.tile([128, 2, HPC * 128], F16, tag=f"kv{e}",
                            name=f"kv{e}") for e in range(NE)]
            xt = [[pw.tile([128, HQ], F16, tag=f"xt{e}_{h}", name=f"xt{e}_{h}")
                   for h in range(QH)] for e in range(NE)]
            nc.sync.dma_start(xt[0][0][0:64, :], xT.ap()[0:64, 0:HQ])
            nc.gpsimd.dma_start(xt[0][0][64:128, :], xT.ap()[64:128, 0:HQ])
            bqa_t = pw.tile([HA, HPC], F32, tag="bqa")
            nc.sync.dma_start(bqa_t[:], bqa.ap())
            bka_t = pw.tile([HA, HPC], F32, tag="bka")
            nc.sync.dma_start(bka_t[:], bka.ap())
            for e in range(NE):
                nc.sync.dma_start(wq_t[e][:],
                                  wq.ap()[e * 128:(e + 1) * 128, :])
                nc.sync.dma_start(kv_t[e][:],
                                  wkv.ap()[e * 128:(e + 1) * 128, :])
                if e > 0:
                    nc.gpsimd.dma_start(
                        xt[e][0][:], xT.ap()[e * 128:(e + 1) * 128, 0:HQ])
            for e in range(NE):
                nc.gpsimd.dma_start(
                    xt[e][1][:], xT.ap()[e * 128:(e + 1) * 128, HQ:S])
            wo_t = []
            for h in range(HPC):
                t = pw.tile([HD, E], BF, tag=f"wo{h}", name=f"wo{h}")
                nc.sync.dma_start(t[:], wo.ap()[h])
                wo_t.append(t)

            # persistent per-head tiles
            qT = [[pw.tile([HA, HQ], F16, tag=f"qT{h}_{q}", name=f"qT{h}_{q}")
                   for q in range(QH)] for h in range(HPC)]
            kT = [[pw.tile([HA, HQ], F16, tag=f"kT{h}_{q}", name=f"kT{h}_{q}")
                   for q in range(QH)] for h in range(HPC)]
            aN = [pw.tile([HD, S], BF, tag=f"aN{h}", name=f"aN{h}")
                  for h in range(HPC)]
            vS = [None, None]

            # ---------- projection units (generators for interleaving) --
            def proj_unit(proj, col0, half, drain):
                """One [*,1024] projection: 12 matmuls + drain; yields
                after each matmul so attention work can interleave."""
                ps = pb.tile([128, HQ], F32, tag="pb", name="ps")
                for e in range(NE):
                    if proj == 0:
                        lhsT = wq_t[e][:, col0:col0 + 128]
                    else:
                        lhsT = kv_t[e][:, proj - 1, col0:col0 + 128]
                    for c2 in range(2):
                        nc.tensor.matmul(
                            ps[:, c2 * 512:(c2 + 1) * 512], lhsT,
                            xt[e][half][:, c2 * 512:(c2 + 1) * 512],
                            start=(e == 0), stop=(e == NE - 1))
                        yield
                drain(ps)
                yield

            def drain_q(h, half):
                def f(ps):
                    nc.scalar.activation(
                        qT[h][half][:], ps[0:HA, :], AF.Identity,
                        bias=bqa_t[:, h:h + 1])
                return f

            def drain_k(h, half):
                def f(ps):
                    nc.scalar.activation(
                        kT[h][half][:], ps[0:HA, :], AF.Identity,
                        bias=bka_t[:, h:h + 1])
                return f

            # vT / vS: V' with a built-in ones row (row HD) per head.
            # vS is split in two 2D tiles so each DMA transpose writes a
            # WHOLE tile -- transposes writing 3D-sliced APs are not
            # tracked correctly against later stationary reads.
            VR = 128  # full stationary width (HW wants M=128)
            vT = [pvt.tile([VR, S], BF, tag=f"vT{h}", name=f"vT{h}")
                  for h in range(HPC)]
            vS = [[pvt.tile([128, NT // 2, VR], BF, tag=f"vS{h}_{p}",
                            name=f"vS{h}_{p}") for p in range(2)]
                  for h in range(HPC)]

            def v_unit(h, half):
                if half == 0:
                    nc.gpsimd.memset(vT[h][HD:VR, :], 0.0)
                    nc.gpsimd.memset(vT[h][HD:HD + 1, :], 1.0)

                def dv(ps):
                    nc.vector.tensor_copy(
                        vT[h][0:HD, half * HQ:(half + 1) * HQ], ps[0:HD, :])
                yield from proj_unit(2, h * 128, half, dv)
                if half == 1:
                    # split transpose: key tiles 0..7 / 8..15
                    nc.sync.dma_start_transpose(vS[h][0][:], vT[h][:, 0:HQ])
                    nc.sync.dma_start_transpose(vS[h][1][:], vT[h][:, HQ:S])
                    yield

            def outproj_unit(qt):
                pf = pb.tile([128, HQ], F32, tag="pb", name="pf")
                if qt % 4 == 0:
                    # fence: moving-operand read of the attN columns this
                    # 512-chunk depends on (written by the normalize mul)
                    for h2 in range(HPC):
                        nc.tensor.matmul(
                            pf[:, 0:1], aN[h2][:, qt * 128:(qt + 1) * 128],
                            aN[h2][:, qt * 128:qt * 128 + 1],
                            start=True, stop=True)
                    yield
                for h2 in range(HPC):
                    lhsT = aN[h2][:, qt * 128:(qt + 1) * 128]
                    nc.tensor.matmul(
                        pf[:, 0:512], lhsT, wo_t[h2][:, 0:512],
                        start=(h2 == 0), stop=(h2 == HPC - 1))
                    yield
                    nc.tensor.matmul(
                        pf[:, 512:768], lhsT, wo_t[h2][:, 512:768],
                        start=(h2 == 0), stop=(h2 == HPC - 1))
                    yield
                ot = pout.tile([128, E], F16, tag="ot", name="ot")
                if qt % 2 == 0:
                    nc.vector.tensor_copy(ot[:], pf[:, 0:E])
                else:
                    nc.scalar.activation(ot[:], pf[:, 0:E], AF.Identity)
                if qt < 8:
                    eng = nc.sync if qt % 2 == 0 else nc.gpsimd
                    eng.dma_start(out.ap()[qt * 128:(qt + 1) * 128, :], ot[:])
                else:
                    # tail tiles: split rows across two issue queues so the
                    # last transfers drain in parallel (per-queue ~23 GB/s)
                    nc.sync.dma_start(
                        out.ap()[qt * 128:qt * 128 + 64, :], ot[0:64, :])
                    nc.gpsimd.dma_start(
                        out.ap()[qt * 128 + 64:(qt + 1) * 128, :], ot[64:128, :])
                yield

            # global ordered filler chain with labels
            def filler_chain():
                yield from v_unit(0, 0)
                yield from proj_unit(1, 0 * 128, 1, drain_k(0, 1))
                yield "k01"
                yield from v_unit(0, 1)
                yield "vS0"
                yield from proj_unit(0, 0 * 128, 1, drain_q(0, 1))
                yield "q01"
                yield from proj_unit(0, 1 * 128, 0, drain_q(1, 0))
                yield from proj_unit(1, 1 * 128, 0, drain_k(1, 0))
                yield "qk10"
                yield from v_unit(1, 0)
                yield from v_unit(1, 1)
                yield "vS1"
                yield from proj_unit(1, 1 * 128, 1, drain_k(1, 1))
                yield "k11"

            fill = filler_chain()
            done_labels = set()

            def feed(n):
                for _ in range(n):
                    for step in fill:
                        if isinstance(step, str):
                            done_labels.add(step)
                            continue
                        break
                    else:
                        return

            def feed_until(label):
                while label not in done_labels:
                    for step in fill:
                        if isinstance(step, str):
                            done_labels.add(step)
                            if step == label:
                                break
                        else:
                            break
                    else:
                        return

            # ---------- warm-up projections (not interleaved) -----------
            for _ in proj_unit(0, 0 * 128, 0, drain_q(0, 0)):
                pass
            for _ in proj_unit(1, 0 * 128, 0, drain_k(0, 0)):
                pass

            def fence_vs(att, h):
                # moving-operand reads carry proper waits; once these two
                # matmuls retire, the in-order PE queue is safe to load
                # vS tiles as stationary weights.
                for p in range(2):
                    nc.tensor.matmul(att[:, 0:1], xt[0][0][:, 0:128],
                                     vS[h][p][:, 0, 0:1],
                                     start=True, stop=True)

            # ---------- attention: 4 passes of 16 key tiles -------------
            def pv_mm(att, h, kt, pt):
                for c in range(2):
                    nc.tensor.matmul(
                        att[:, c * 512:(c + 1) * 512],
                        vS[h][kt // 8][:, kt % 8, :],
                        pt[:, c * 512:(c + 1) * 512],
                        start=(kt == 0), stop=(kt == NT - 1))

            def attention_pass(h, qh, fpk, interleave_pv, pv_prereq=None,
                               kt8_prereq=None, post=None, dbg_cap=False):
                att = pa.tile([128, HQ], F32, tag="att", name="att")
                if interleave_pv and qh == 0:
                    fence_vs(att, h)
                pts = []
                for kt in range(NT):
                    if kt == 8 and kt8_prereq is not None:
                        # the second-half K tile must be EMITTED before any
                        # score matmul that reads it
                        feed_until(kt8_prereq)
                    sc = pb.tile([128, HQ], F32, tag="pb", name="sc")
                    kTt = kT[h][kt // 8]
                    k0 = (kt % 8) * 128
                    for c in range(2):
                        nc.tensor.matmul(
                            sc[:, c * 512:(c + 1) * 512],
                            kTt[:, k0:k0 + 128],
                            qT[h][qh][:, c * 512:(c + 1) * 512],
                            start=True, stop=True)
                    if interleave_pv and kt > 0:
                        pv_mm(att, h, kt - 1, pts[-1])
                    feed(fpk[kt] if isinstance(fpk, list) else fpk)
                    pt = ppr.tile([128, HQ], BF, tag="probsT", name="pt")
                    nc.scalar.activation(pt[:], sc[:], AF.Exp)
                    if dbg_cap and kt == 0 and _CACHE.get("debug"):
                        capp = pw.tile([128, HQ], F32, tag="dbgpt", name="capp")
                        nc.vector.tensor_copy(capp[:], pt[:])
                        nc.sync.dma_start(dbg["pt0"].ap(), capp[:])
                    pts.append(pt)
                if interleave_pv:
                    pv_mm(att, h, NT - 1, pts[-1])
                else:
                    if pv_prereq is not None:
                        feed_until(pv_prereq)
                    fence_vs(att, h)
                    for kt in range(NT):
                        pv_mm(att, h, kt, pts[kt])
                if dbg_cap and _CACHE.get("debug"):
                    cap = pw.tile([128, HQ], F32, tag="dbgcap", name="cap")
                    nc.vector.tensor_copy(cap[:], att[:])
                    nc.sync.dma_start(dbg["att00"].ap(), cap[:])
                # normalize in 512-col chunks.  The sums row must bounce
                # through SBUF (reciprocal reading PSUM directly returns
                # garbage on HW); the copy runs on the scalar engine,
                # which is idle once the pass's exps are done.  post(c)
                # emits dependent work (the matching output-projection
                # tiles) right after chunk c's multiply.
                for c in range(2):
                    cs = slice(c * 512, (c + 1) * 512)
                    sR = pno.tile([1, 512], F32, tag=f"sR{c}", name=f"sR{c}")
                    nc.vector.tensor_copy(sR[:], att[HD:HD + 1, cs])
                    rR = pno.tile([1, 512], F32, tag=f"rR{c}", name=f"rR{c}")
                    nc.vector.reciprocal_approx_fast(rR[:], sR[:])
                    rb = pno.tile([HD, 512], F32, tag=f"rb{c}", name=f"rb{c}")
                    nc.gpsimd.partition_broadcast(rb[:], rR[:])
                    nc.vector.tensor_mul(
                        aN[h][:, qh * HQ + c * 512:qh * HQ + (c + 1) * 512],
                        att[0:HD, cs], rb[:])
                    if post is not None:
                        post(c)

            def op_range(q0, q1):
                for qt in range(q0, q1):
                    for _ in outproj_unit(qt):
                        pass

            # pass (0,0): vS0 is emitted mid-pass by fillers, so PVs are
            # emitted as a block at the end (needs all 16 pt tiles live).
            attention_pass(0, 0, 3, interleave_pv=False, pv_prereq="vS0",
                           kt8_prereq="k01", dbg_cap=True)
            feed_until("q01")
            attention_pass(0, 1, 3, interleave_pv=True)
            feed_until("qk10")
            feed_until("vS1")
            attention_pass(1, 0, 3, interleave_pv=True, kt8_prereq="k11")
            feed(10000)  # flush any remaining projection fillers
            # q1's second-half Q projection runs here: it hides the head-1
            # qh0 normalize latency before the first output-projection block
            for _ in proj_unit(0, 1 * 128, 1, drain_q(1, 1)):
                pass
            op_range(0, 4)

            def post11(c):
                if c == 0:
                    # qt 4..7 are ready (normalized long ago); they hide
                    # the first normalize chunk's latency
                    op_range(4, 8)
                    op_range(8, 12)
                else:
                    op_range(12, 16)

            attention_pass(1, 1, 0, interleave_pv=True, post=post11)

            if _CACHE.get("debug"):
                nc.sync.dma_start(dbg["qT00"].ap(), qT[0][0][:])
                nc.sync.dma_start(dbg["kT00"].ap(), kT[0][0][:])
                nc.sync.dma_start(dbg["kT01"].ap(), kT[0][1][:])
                nc.sync.dma_start(dbg["vS0"].ap()[:, 0:8, :], vS[0][0][:])
                nc.sync.dma_start(dbg["vS0"].ap()[:, 8:16, :], vS[0][1][:])
                nc.sync.dma_start(dbg["aN0"].ap(), aN[0][:])
                nc.sync.dma_start(dbg["aN1"].ap(), aN[1][:])

    nc.compile()
    return nc


def kernel(x, Wq, bq, Wk, bk, Wv, bv, Wo, bo):
    x = np.asarray(x, np.float32)
    Wq, bq = np.asarray(Wq, np.float32), np.asarray(bq, np.float32)
    Wk, bk = np.asarray(Wk, np.float32), np.asarray(bk, np.float32)
    Wv, bv = np.asarray(Wv, np.float32), np.asarray(bv, np.float32)
    Wo, bo = np.asarray(Wo, np.float32), np.asarray(bo, np.float32)

    if "nc" not in _CACHE:
        _CACHE["nc"] = _build()
    nc = _CACHE["nc"]

    bo_p = bo.astype(np.float64) + SCALING * (bv.astype(np.float64)
                                              @ Wo.astype(np.float64))

    in_maps = []
    for core in range(N_CORES):
        b = core // 4
        h0 = (core % 4) * HPC
        wq_a = np.zeros((E, HPC, 128), np.float32)
        wk_a = np.zeros((E, HPC, 128), np.float32)
        wv_s = np.zeros((E, HPC, 128), np.float32)
        wo_s = np.zeros((HPC, HD, E), np.float32)
        bqa = np.zeros((HA, HPC), np.float32)
        bka = np.zeros((HA, HPC), np.float32)
        for j in range(HPC):
            sl = slice((h0 + j) * HD, (h0 + j + 1) * HD)
            wq_a[:, j, 0:HD] = Wq[:, sl]
            wq_a[:, j, HD] = Wq[:, sl] @ bk[sl]
            # wq_a[:, j, HD+1] stays 0 (constant 1 comes from the bias)
            wk_a[:, j, 0:HD] = Wk[:, sl]
            # wk_a[:, j, HD] stays 0 (constant 1 via bias)
            wk_a[:, j, HD + 1] = Wk[:, sl] @ bq[sl]
            wv_s[:, j, 0:HD] = Wv[:, sl]
            wo_s[j] = SCALING * Wo[sl, :]
            bqa[HD, j] = float(bq[sl] @ bk[sl])
            bqa[HD + 1, j] = 1.0
            bka[HD, j] = 1.0
        wkv = np.stack([wk_a.reshape(E, HPC * 128),
                        wv_s.reshape(E, HPC * 128)], axis=1)
        in_maps.append({
            "xT": np.ascontiguousarray(x[b].T).astype(np.float16),
            "wq": wq_a.reshape(E, HPC * 128).astype(np.float16),
            "wkv": np.ascontiguousarray(wkv).reshape(
                E, 2 * HPC * 128).astype(np.float16),
            "wo": wo_s.astype(bf16),
            "bqa": bqa,
            "bka": bka,
        })

    res = bass_utils.run_bass_kernel_spmd(
        nc, in_maps, core_ids=list(range(N_CORES)))
    _CACHE["last_result"] = res

    parts = [res.results[i]["out"].astype(np.float64) for i in range(N_CORES)]
    full = np.stack([sum(parts[b * 4:(b + 1) * 4]) + bo_p for b in range(B)])
    return full.astype(np.float32)


# revision 31
# speedup vs baseline: 1.0080x; 1.0080x over previous
"""Multi-head attention (B=2, S=2048, E=768, H=8) on 8 Trainium2 NeuronCores.

Sharding: core i handles batch b = i//4 and heads {2*(i%4), 2*(i%4)+1}
(data parallel on B, tensor parallel on heads). Each core computes its
QKV projections (column-sliced weights), full attention for its 2 heads,
and a partial output projection (row-sliced Wo). The host sums the 4
partials per batch and adds the (adjusted) output bias.

Schedule (all PE work in fp16/bf16 at full rate, 2 cols/cycle):
- q/k biases are folded exactly into two augmented weight columns
  (head dim 96 -> 98), so energies = (xWq+bq)(xWk+bk)^T come out of a
  single matmul over fp16 q~/k~ tiles.
- Attention runs in 4 passes (2 heads x 2 query halves of 1024).  Each
  pass: per key tile kt, scores (2 matmuls) -> exp on the ACT engine ->
  PV (2 matmuls, accumulated in a [128,1024] PSUM tile).  The exp
  (1.1us per [128,1024]) paces the loop, so all remaining projection
  matmuls are interleaved as fillers to keep the PE dense.
- V' carries a built-in ones row (row 96 of vT), so the PV stationary
  slice [*, 0:97] directly yields the softmax row sums in partition 96
  with no copies or memsets of a separate V buffer.
- Normalization reads the PV PSUM directly: reciprocal of row 96,
  gpsimd partition-broadcast, one vector multiply -> bf16 attN.
- Output projection accumulates both heads per 128-query tile in PSUM,
  drains via vector cast to fp16, and streams fp16 tiles to HBM
  (halves the output DMA vs f32).  The host sums partials in f64.
- The V bias contributes a constant (softmax rows sum to 1), folded
  into bo on the host: bo' = bo + scaling * (bv @ Wo). The softmax
  "scaling" quirk is folded into Wo' = scaling * Wo.
"""

import numpy as np
import ml_dtypes

import concourse.mybir as mybir
import concourse.tile as tile
from concourse import bacc
from concourse import bass_utils

bf16 = ml_dtypes.bfloat16
F32 = mybir.dt.float32
BF = mybir.dt.bfloat16
F16 = mybir.dt.float16
AF = mybir.ActivationFunctionType

B, S, E, H, HD = 2, 2048, 768, 8, 96
HA = HD + 2          # augmented head dim (bias folding)
HPC = 2              # heads per core
N_CORES = 8
SCALING = HD ** -0.5
NE = E // 128        # 6 contraction tiles for projections
NT = S // 128        # 16 key tiles
QH = 2               # query halves of 1024
HQ = S // QH         # 1024

_CACHE = {}


def _build():
    nc = bacc.Bacc("TRN2", target_bir_lowering=False, debug=False,
                   enable_asserts=False, num_devices=N_CORES)

    xT = nc.dram_tensor("xT", [E, S], F16, kind="ExternalInput")
    wq = nc.dram_tensor("wq", [E, HPC * 128], F16, kind="ExternalInput")
    wkv = nc.dram_tensor("wkv", [E, 2 * HPC * 128], F16,
                         kind="ExternalInput")
    wo = nc.dram_tensor("wo", [HPC, HD, E], BF, kind="ExternalInput")
    bqa = nc.dram_tensor("bqa", [HA, HPC], F32, kind="ExternalInput")
    bka = nc.dram_tensor("bka", [HA, HPC], F32, kind="ExternalInput")
    out = nc.dram_tensor("out", [S, E], F16, kind="ExternalOutput")
    dbg = {}
    if _CACHE.get("debug"):
        dbg["qT00"] = nc.dram_tensor("d_qT00", [HA, HQ], F16,
                                     kind="ExternalOutput")
        dbg["kT00"] = nc.dram_tensor("d_kT00", [HA, HQ], F16,
                                     kind="ExternalOutput")
        dbg["kT01"] = nc.dram_tensor("d_kT01", [HA, HQ], F16,
                                     kind="ExternalOutput")
        dbg["vS0"] = nc.dram_tensor("d_vS0", [128, 16, 128], BF,
                                    kind="ExternalOutput")
        dbg["aN0"] = nc.dram_tensor("d_aN0", [HD, S], BF,
                                    kind="ExternalOutput")
        dbg["aN1"] = nc.dram_tensor("d_aN1", [HD, S], BF,
                                    kind="ExternalOutput")
        dbg["att00"] = nc.dram_tensor("d_att00", [128, HQ], F32,
                                      kind="ExternalOutput")
        dbg["pt0"] = nc.dram_tensor("d_pt0", [128, HQ], F32,
                                    kind="ExternalOutput")
        dbg["rb0"] = nc.dram_tensor("d_rb0", [HD, 512], F32,
                                    kind="ExternalOutput")

    with tile.TileContext(nc) as tc:
        with tc.tile_pool(name="pw", bufs=1) as pw, \
             tc.tile_pool(name="pvt", bufs=4) as pvt, \
             tc.tile_pool(name="ppr", bufs=18) as ppr, \
             tc.tile_pool(name="pno", bufs=2) as pno, \
             tc.tile_pool(name="pout", bufs=4) as pout, \
             tc.tile_pool(name="pa", bufs=1, space="PSUM") as pa, \
             tc.tile_pool(name="pb", bufs=3, space="PSUM") as pb:

            # ---------- input DMAs ----------
            # wq tiles stay small (fast first arrival); k/v weights merge
            # into one [128, 2, 256] tile per e-block.  x half tiles are on
            # the gpsimd issue queue, with the first one split across both
            # queues.  Each dma_start costs ~650ns of serial issue time.
            wq_t = [pw.tile([128, HPC * 128], F16, tag=f"wq{e}",
                            name=f"wq{e}") for e in range(NE)]
            kv_t = [pw.tile([128, 2, HPC * 128], F16, tag=f"kv{e}",
                            name=f"kv{e}") for e in range(NE)]
            xt = [[pw.tile([128, HQ], F16, tag=f"xt{e}_{h}", name=f"xt{e}_{h}")
                   for h in range(QH)] for e in range(NE)]
            nc.sync.dma_start(xt[0][0][0:64, :], xT.ap()[0:64, 0:HQ])
            nc.gpsimd.dma_start(xt[0][0][64:128, :], xT.ap()[64:128, 0:HQ])
            for e in range(1, NE):
                nc.gpsimd.dma_start(
                    xt[e][0][:], xT.ap()[e * 128:(e + 1) * 128, 0:HQ])

            def sync_dma(t, ap):
                nc.sync.dma_start(t[:], ap)
            kv_ap = [wkv.ap()[e * 128:(e + 1) * 128, :] for e in range(NE)]
            for e in range(4):
                sync_dma(wq_t[e], wq.ap()[e * 128:(e + 1) * 128, :])
            sync_dma(kv_t[0], kv_ap[0])
            sync_dma(wq_t[4], wq.ap()[4 * 128:5 * 128, :])
            sync_dma(kv_t[1], kv_ap[1])
            sync_dma(wq_t[5], wq.ap()[5 * 128:6 * 128, :])
            sync_dma(kv_t[2], kv_ap[2])
            bqa_t = pw.tile([HA, HPC], F32, tag="bqa")
            nc.sync.dma_start(bqa_t[:], bqa.ap())
            bka_t = pw.tile([HA, HPC], F32, tag="bka")
            nc.sync.dma_start(bka_t[:], bka.ap())
            for e in range(3, NE):
                sync_dma(kv_t[e], kv_ap[e])
            for e in range(NE):
                nc.gpsimd.dma_start(
                    xt[e][1][:], xT.ap()[e * 128:(e + 1) * 128, HQ:S])
            wo_t = []
            for h in range(HPC):
                t = pw.tile([HD, E], BF, tag=f"wo{h}", name=f"wo{h}")
                nc.sync.dma_start(t[:], wo.ap()[h])
                wo_t.append(t)

            # persistent per-head tiles
            qT = [[pw.tile([HA, HQ], F16, tag=f"qT{h}_{q}", name=f"qT{h}_{q}")
                   for q in range(QH)] for h in range(HPC)]
            kT = [[pw.tile([HA, HQ], F16, tag=f"kT{h}_{q}", name=f"kT{h}_{q}")
                   for q in range(QH)] for h in range(HPC)]
            aN = [pw.tile([HD, S], BF, tag=f"aN{h}", name=f"aN{h}")
                  for h in range(HPC)]
            vS = [None, None]

            # ---------- projection units (generators for interleaving) --
            def proj_unit(proj, col0, half, drain):
                """One [*,1024] projection: 12 matmuls + drain; yields
                after each matmul so attention work can interleave."""
                ps = pb.tile([128, HQ], F32, tag="pb", name="ps")
                for e in range(NE):
                    if proj == 0:
                        lhsT = wq_t[e][:, col0:col0 + 128]
                    else:
                        lhsT = kv_t[e][:, proj - 1, col0:col0 + 128]
                    for c2 in range(2):
                        nc.tensor.matmul(
                            ps[:, c2 * 512:(c2 + 1) * 512], lhsT,
                            xt[e][half][:, c2 * 512:(c2 + 1) * 512],
                            start=(e == 0), stop=(e == NE - 1))
                        yield
                drain(ps)
                yield

            def drain_q(h, half):
                def f(ps):
                    nc.scalar.activation(
                        qT[h][half][:], ps[0:HA, :], AF.Identity,
                        bias=bqa_t[:, h:h + 1])
                return f

            def drain_k(h, half):
                def f(ps):
                    nc.scalar.activation(
                        kT[h][half][:], ps[0:HA, :], AF.Identity,
                        bias=bka_t[:, h:h + 1])
                return f

            # vT / vS: V' with a built-in ones row (row HD) per head.
            # vS is split in two 2D tiles so each DMA transpose writes a
            # WHOLE tile -- transposes writing 3D-sliced APs are not
            # tracked correctly against later stationary reads.
            VR = 128  # full stationary width (HW wants M=128)
            vT = [pvt.tile([VR, S], BF, tag=f"vT{h}", name=f"vT{h}")
                  for h in range(HPC)]
            vS = [[pvt.tile([128, NT // 2, VR], BF, tag=f"vS{h}_{p}",
                            name=f"vS{h}_{p}") for p in range(2)]
                  for h in range(HPC)]

            def v_unit(h, half):
                if half == 0:
                    nc.gpsimd.memset(vT[h][HD:VR, :], 0.0)
                    nc.gpsimd.memset(vT[h][HD:HD + 1, :], 1.0)

                def dv(ps):
                    nc.vector.tensor_copy(
                        vT[h][0:HD, half * HQ:(half + 1) * HQ], ps[0:HD, :])
                yield from proj_unit(2, h * 128, half, dv)
                if half == 1:
                    # split transpose: key tiles 0..7 / 8..15
                    nc.sync.dma_start_transpose(vS[h][0][:], vT[h][:, 0:HQ])
                    nc.sync.dma_start_transpose(vS[h][1][:], vT[h][:, HQ:S])
                    yield

            def outproj_unit(qt):
                pf = pb.tile([128, HQ], F32, tag="pb", name="pf")
                if qt % 4 == 0:
                    # fence: moving-operand read of the attN columns this
                    # 512-chunk depends on (written by the normalize mul)
                    for h2 in range(HPC):
                        nc.tensor.matmul(
                            pf[:, 0:1], aN[h2][:, qt * 128:(qt + 1) * 128],
                            aN[h2][:, qt * 128:qt * 128 + 1],
                            start=True, stop=True)
                    yield
                for h2 in range(HPC):
                    lhsT = aN[h2][:, qt * 128:(qt + 1) * 128]
                    nc.tensor.matmul(
                        pf[:, 0:512], lhsT, wo_t[h2][:, 0:512],
                        start=(h2 == 0), stop=(h2 == HPC - 1))
                    yield
                    nc.tensor.matmul(
                        pf[:, 512:768], lhsT, wo_t[h2][:, 512:768],
                        start=(h2 == 0), stop=(h2 == HPC - 1))
                    yield
                ot = pout.tile([128, E], F16, tag="ot", name="ot")
                if qt % 2 == 0:
                    nc.vector.tensor_copy(ot[:], pf[:, 0:E])
                else:
                    nc.scalar.activation(ot[:], pf[:, 0:E], AF.Identity)
                if qt < 8:
                    eng = nc.sync if qt % 2 == 0 else nc.gpsimd
                    eng.dma_start(out.ap()[qt * 128:(qt + 1) * 128, :], ot[:])
                else:
                    # tail tiles: split rows across two issue queues so the
                    # last transfers drain in parallel (per-queue ~23 GB/s)
                    nc.sync.dma_start(
                        out.ap()[qt * 128:qt * 128 + 64, :], ot[0:64, :])
                    nc.gpsimd.dma_start(
                        out.ap()[qt * 128 + 64:(qt + 1) * 128, :], ot[64:128, :])
                yield

            # global ordered filler chain with labels
            def filler_chain():
                yield from v_unit(0, 0)
                yield from proj_unit(1, 0 * 128, 1, drain_k(0, 1))
                yield "k01"
                yield from v_unit(0, 1)
                yield "vS0"
                yield from proj_unit(0, 0 * 128, 1, drain_q(0, 1))
                yield "q01"
                yield from proj_unit(0, 1 * 128, 0, drain_q(1, 0))
                yield from proj_unit(1, 1 * 128, 0, drain_k(1, 0))
                yield "qk10"
                yield from v_unit(1, 0)
                yield from v_unit(1, 1)
                yield "vS1"
                yield from proj_unit(1, 1 * 128, 1, drain_k(1, 1))
                yield "k11"

            fill = filler_chain()
            done_labels = set()

            def feed(n):
                for _ in range(n):
                    for step in fill:
                        if isinstance(step, str):
                            done_labels.add(step)
                            continue
                        break
                    else:
                        return

            def feed_until(label):
                while label not in done_labels:
                    for step in fill:
                        if isinstance(step, str):
                            done_labels.add(step)
                            if step == label:
                                break
                        else:
                            break
                    else:
                        return

            # ---------- warm-up projections (not interleaved) -----------
            for _ in proj_unit(0, 0 * 128, 0, drain_q(0, 0)):
                pass
            for _ in proj_unit(1, 0 * 128, 0, drain_k(0, 0)):
                pass

            def fence_vs(att, h):
                # moving-operand reads carry proper waits; once these two
                # matmuls retire, the in-order PE queue is safe to load
                # vS tiles as stationary weights.
                for p in range(2):
                    nc.tensor.matmul(att[:, 0:1], xt[0][0][:, 0:128],
                                     vS[h][p][:, 0, 0:1],
                                     start=True, stop=True)

            # ---------- attention: 4 passes of 16 key tiles -------------
            def pv_mm(att, h, kt, pt):
                for c in range(2):
                    nc.tensor.matmul(
                        att[:, c * 512:(c + 1) * 512],
                        vS[h][kt // 8][:, kt % 8, :],
                        pt[:, c * 512:(c + 1) * 512],
                        start=(kt == 0), stop=(kt == NT - 1))

            def attention_pass(h, qh, fpk, interleave_pv, pv_prereq=None,
                               kt8_prereq=None, post=None, dbg_cap=False):
                att = pa.tile([128, HQ], F32, tag="att", name="att")
                if interleave_pv and qh == 0:
                    fence_vs(att, h)
                pts = []
                for kt in range(NT):
                    if kt == 8 and kt8_prereq is not None:
                        # the second-half K tile must be EMITTED before any
                        # score matmul that reads it
                        feed_until(kt8_prereq)
                    sc = pb.tile([128, HQ], F32, tag="pb", name="sc")
                    kTt = kT[h][kt // 8]
                    k0 = (kt % 8) * 128
                    for c in range(2):
                        nc.tensor.matmul(
                            sc[:, c * 512:(c + 1) * 512],
                            kTt[:, k0:k0 + 128],
                            qT[h][qh][:, c * 512:(c + 1) * 512],
                            start=True, stop=True)
                    if interleave_pv and kt > 0:
                        pv_mm(att, h, kt - 1, pts[-1])
                    feed(fpk[kt] if isinstance(fpk, list) else fpk)
                    pt = ppr.tile([128, HQ], BF, tag="probsT", name="pt")
                    nc.scalar.activation(pt[:], sc[:], AF.Exp)
                    if dbg_cap and kt == 0 and _CACHE.get("debug"):
                        capp = pw.tile([128, HQ], F32, tag="dbgpt", name="capp")
                        nc.vector.tensor_copy(capp[:], pt[:])
                        nc.sync.dma_start(dbg["pt0"].ap(), capp[:])
                    pts.append(pt)
                if interleave_pv:
                    pv_mm(att, h, NT - 1, pts[-1])
                else:
                    if pv_prereq is not None:
                        feed_until(pv_prereq)
                    fence_vs(att, h)
                    for kt in range(NT):
                        pv_mm(att, h, kt, pts[kt])
                if dbg_cap and _CACHE.get("debug"):
                    cap = pw.tile([128, HQ], F32, tag="dbgcap", name="cap")
                    nc.vector.tensor_copy(cap[:], att[:])
                    nc.sync.dma_start(dbg["att00"].ap(), cap[:])
                # normalize in 512-col chunks.  The sums row must bounce
                # through SBUF (reciprocal reading PSUM directly returns
                # garbage on HW); the copy runs on the scalar engine,
                # which is idle once the pass's exps are done.  post(c)
                # emits dependent work (the matching output-projection
                # tiles) right after chunk c's multiply.
                for c in range(2):
                    cs = slice(c * 512, (c + 1) * 512)
                    sR = pno.tile([1, 512], F32, tag=f"sR{c}", name=f"sR{c}")
                    nc.vector.tensor_copy(sR[:], att[HD:HD + 1, cs])
                    rR = pno.tile([1, 512], F32, tag=f"rR{c}", name=f"rR{c}")
                    nc.vector.reciprocal_approx_fast(rR[:], sR[:])
                    rb = pno.tile([HD, 512], F32, tag=f"rb{c}", name=f"rb{c}")
                    nc.gpsimd.partition_broadcast(rb[:], rR[:])
                    nc.vector.tensor_mul(
                        aN[h][:, qh * HQ + c * 512:qh * HQ + (c + 1) * 512],
                        att[0:HD, cs], rb[:])
                    if post is not None:
                        post(c)

            def op_range(q0, q1):
                for qt in range(q0, q1):
                    for _ in outproj_unit(qt):
                        pass

            # pass (0,0): vS0 is emitted mid-pass by fillers, so PVs are
            # emitted as a block at the end (needs all 16 pt tiles live).
            attention_pass(0, 0, 3, interleave_pv=False, pv_prereq="vS0",
                           kt8_prereq="k01", dbg_cap=True)
            feed_until("q01")
            attention_pass(0, 1, 3, interleave_pv=True)
            feed_until("qk10")
            feed_until("vS1")
            attention_pass(1, 0, 3, interleave_pv=True, kt8_prereq="k11")
            feed(10000)  # flush any remaining projection fillers
            # q1's second-half Q projection runs here: it hides the head-1
            # qh0 normalize latency before the first output-projection block
            for _ in proj_unit(0, 1 * 128, 1, drain_q(1, 1)):
                pass
            op_range(0, 4)

            def post11(c):
                if c == 0:
                    # qt 4..7 are ready (normalized long ago); they hide
                    # the first normalize chunk's latency
                    op_range(4, 8)
                    op_range(8, 12)
                else:
                    op_range(12, 16)

            attention_pass(1, 1, 0, interleave_pv=True, post=post11)

            if _CACHE.get("debug"):
                nc.sync.dma_start(dbg["qT00"].ap(), qT[0][0][:])
                nc.sync.dma_start(dbg["kT00"].ap(), kT[0][0][:])
                nc.sync.dma_start(dbg["kT01"].ap(), kT[0][1][:])
                nc.sync.dma_start(dbg["vS0"].ap()[:, 0:8, :], vS[0][0][:])
                nc.sync.dma_start(dbg["vS0"].ap()[:, 8:16, :], vS[0][1][:])
                nc.sync.dma_start(dbg["aN0"].ap(), aN[0][:])
                nc.sync.dma_start(dbg["aN1"].ap(), aN[1][:])

    nc.compile()
    return nc


def kernel(x, Wq, bq, Wk, bk, Wv, bv, Wo, bo):
    x = np.asarray(x, np.float32)
    Wq, bq = np.asarray(Wq, np.float32), np.asarray(bq, np.float32)
    Wk, bk = np.asarray(Wk, np.float32), np.asarray(bk, np.float32)
    Wv, bv = np.asarray(Wv, np.float32), np.asarray(bv, np.float32)
    Wo, bo = np.asarray(Wo, np.float32), np.asarray(bo, np.float32)

    if "nc" not in _CACHE:
        _CACHE["nc"] = _build()
    nc = _CACHE["nc"]

    bo_p = bo.astype(np.float64) + SCALING * (bv.astype(np.float64)
                                              @ Wo.astype(np.float64))

    in_maps = []
    for core in range(N_CORES):
        b = core // 4
        h0 = (core % 4) * HPC
        wq_a = np.zeros((E, HPC, 128), np.float32)
        wk_a = np.zeros((E, HPC, 128), np.float32)
        wv_s = np.zeros((E, HPC, 128), np.float32)
        wo_s = np.zeros((HPC, HD, E), np.float32)
        bqa = np.zeros((HA, HPC), np.float32)
        bka = np.zeros((HA, HPC), np.float32)
        for j in range(HPC):
            sl = slice((h0 + j) * HD, (h0 + j + 1) * HD)
            wq_a[:, j, 0:HD] = Wq[:, sl]
            wq_a[:, j, HD] = Wq[:, sl] @ bk[sl]
            # wq_a[:, j, HD+1] stays 0 (constant 1 comes from the bias)
            wk_a[:, j, 0:HD] = Wk[:, sl]
            # wk_a[:, j, HD] stays 0 (constant 1 via bias)
            wk_a[:, j, HD + 1] = Wk[:, sl] @ bq[sl]
            wv_s[:, j, 0:HD] = Wv[:, sl]
            wo_s[j] = SCALING * Wo[sl, :]
            bqa[HD, j] = float(bq[sl] @ bk[sl])
            bqa[HD + 1, j] = 1.0
            bka[HD, j] = 1.0
        wkv = np.stack([wk_a.reshape(E, HPC * 128),
                        wv_s.reshape(E, HPC * 128)], axis=1)
        in_maps.append({
            "xT": np.ascontiguousarray(x[b].T).astype(np.float16),
            "wq": wq_a.reshape(E, HPC * 128).astype(np.float16),
            "wkv": np.ascontiguousarray(wkv).reshape(
                E, 2 * HPC * 128).astype(np.float16),
            "wo": wo_s.astype(bf16),
            "bqa": bqa,
            "bka": bka,
        })

    res = bass_utils.run_bass_kernel_spmd(
        nc, in_maps, core_ids=list(range(N_CORES)))
    _CACHE["last_result"] = res

    parts = [res.results[i]["out"].astype(np.float64) for i in range(N_CORES)]
    full = np.stack([sum(parts[b * 4:(b + 1) * 4]) + bo_p for b in range(B)])
    return full.astype(np.float32)


# revision 32
# speedup vs baseline: 1.0329x; 1.0248x over previous
"""Multi-head attention (B=2, S=2048, E=768, H=8) on 8 Trainium2 NeuronCores.

Sharding: core i handles batch b = i//4 and heads {2*(i%4), 2*(i%4)+1}
(data parallel on B, tensor parallel on heads). Each core computes its
QKV projections (column-sliced weights), full attention for its 2 heads,
and a partial output projection (row-sliced Wo). The host sums the 4
partials per batch and adds the (adjusted) output bias.

Schedule (all PE work in fp16/bf16 at full rate, 2 cols/cycle):
- q/k biases are folded exactly into two augmented weight columns
  (head dim 96 -> 98), so energies = (xWq+bq)(xWk+bk)^T come out of a
  single matmul over fp16 q~/k~ tiles.
- Attention runs in 4 passes (2 heads x 2 query halves of 1024).  Each
  pass: per key tile kt, scores (2 matmuls) -> exp on the ACT engine ->
  PV (2 matmuls, accumulated in a [128,1024] PSUM tile).  The exp
  (1.1us per [128,1024]) paces the loop, so all remaining projection
  matmuls are interleaved as fillers to keep the PE dense.
- V' carries a built-in ones row (row 96 of vT), so the PV stationary
  slice [*, 0:97] directly yields the softmax row sums in partition 96
  with no copies or memsets of a separate V buffer.
- Normalization reads the PV PSUM directly: reciprocal of row 96,
  gpsimd partition-broadcast, one vector multiply -> bf16 attN.
- Output projection accumulates both heads per 128-query tile in PSUM,
  drains via vector cast to fp16, and streams fp16 tiles to HBM
  (halves the output DMA vs f32).  The host sums partials in f64.
- The V bias contributes a constant (softmax rows sum to 1), folded
  into bo on the host: bo' = bo + scaling * (bv @ Wo). The softmax
  "scaling" quirk is folded into Wo' = scaling * Wo.
"""

import numpy as np
import ml_dtypes

import concourse.mybir as mybir
import concourse.tile as tile
from concourse import bacc
from concourse import bass_utils

bf16 = ml_dtypes.bfloat16
F32 = mybir.dt.float32
BF = mybir.dt.bfloat16
F16 = mybir.dt.float16
AF = mybir.ActivationFunctionType

B, S, E, H, HD = 2, 2048, 768, 8, 96
HA = HD + 2          # augmented head dim (bias folding)
HPC = 2              # heads per core
N_CORES = 8
SCALING = HD ** -0.5
NE = E // 128        # 6 contraction tiles for projections
NT = S // 128        # 16 key tiles
QH = 2               # query halves of 1024
HQ = S // QH         # 1024

_CACHE = {}


def _build():
    nc = bacc.Bacc("TRN2", target_bir_lowering=False, debug=False,
                   enable_asserts=False, num_devices=N_CORES)

    xT = nc.dram_tensor("xT", [E, S], F16, kind="ExternalInput")
    wq = nc.dram_tensor("wq", [E, HPC * 128], F16, kind="ExternalInput")
    wkv = nc.dram_tensor("wkv", [E, 2 * HPC * 128], F16,
                         kind="ExternalInput")
    wo = nc.dram_tensor("wo", [HPC, HD, E], BF, kind="ExternalInput")
    bqa = nc.dram_tensor("bqa", [HA, HPC], F32, kind="ExternalInput")
    bka = nc.dram_tensor("bka", [HA, HPC], F32, kind="ExternalInput")
    out = nc.dram_tensor("out", [S, E], F16, kind="ExternalOutput")
    dbg = {}
    if _CACHE.get("debug"):
        dbg["qT00"] = nc.dram_tensor("d_qT00", [HA, HQ], F16,
                                     kind="ExternalOutput")
        dbg["kT00"] = nc.dram_tensor("d_kT00", [HA, HQ], F16,
                                     kind="ExternalOutput")
        dbg["kT01"] = nc.dram_tensor("d_kT01", [HA, HQ], F16,
                                     kind="ExternalOutput")
        dbg["vS0"] = nc.dram_tensor("d_vS0", [128, 16, 128], BF,
                                    kind="ExternalOutput")
        dbg["aN0"] = nc.dram_tensor("d_aN0", [HD, S], BF,
                                    kind="ExternalOutput")
        dbg["aN1"] = nc.dram_tensor("d_aN1", [HD, S], BF,
                                    kind="ExternalOutput")
        dbg["att00"] = nc.dram_tensor("d_att00", [128, HQ], F32,
                                      kind="ExternalOutput")
        dbg["pt0"] = nc.dram_tensor("d_pt0", [128, HQ], F32,
                                    kind="ExternalOutput")
        dbg["rb0"] = nc.dram_tensor("d_rb0", [HD, 512], F32,
                                    kind="ExternalOutput")

    with tile.TileContext(nc) as tc:
        with tc.tile_pool(name="pw", bufs=1) as pw, \
             tc.tile_pool(name="pvt", bufs=4) as pvt, \
             tc.tile_pool(name="ppr", bufs=18) as ppr, \
             tc.tile_pool(name="pno", bufs=2) as pno, \
             tc.tile_pool(name="pout", bufs=4) as pout, \
             tc.tile_pool(name="pa", bufs=1, space="PSUM") as pa, \
             tc.tile_pool(name="pb", bufs=3, space="PSUM") as pb:

            # ---------- input DMAs ----------
            # wq tiles stay small (fast first arrival); k/v weights merge
            # into one [128, 2, 256] tile per e-block.  x half tiles are on
            # the gpsimd issue queue, with the first one split across both
            # queues.  Each dma_start costs ~650ns of serial issue time.
            wq_t = [pw.tile([128, HPC * 128], F16, tag=f"wq{e}",
                            name=f"wq{e}") for e in range(NE)]
            kv_t = [pw.tile([128, 2, HPC * 128], F16, tag=f"kv{e}",
                            name=f"kv{e}") for e in range(NE)]
            xt = [[pw.tile([128, HQ], F16, tag=f"xt{e}_{h}", name=f"xt{e}_{h}")
                   for h in range(QH)] for e in range(NE)]
            nc.sync.dma_start(xt[0][0][0:64, :], xT.ap()[0:64, 0:HQ])
            nc.gpsimd.dma_start(xt[0][0][64:128, :], xT.ap()[64:128, 0:HQ])
            for e in range(1, NE):
                nc.gpsimd.dma_start(
                    xt[e][0][:], xT.ap()[e * 128:(e + 1) * 128, 0:HQ])

            def sync_dma(t, ap):
                nc.sync.dma_start(t[:], ap)
            kv_ap = [wkv.ap()[e * 128:(e + 1) * 128, :] for e in range(NE)]
            for e in range(4):
                sync_dma(wq_t[e], wq.ap()[e * 128:(e + 1) * 128, :])
            sync_dma(kv_t[0], kv_ap[0])
            sync_dma(wq_t[4], wq.ap()[4 * 128:5 * 128, :])
            sync_dma(kv_t[1], kv_ap[1])
            sync_dma(wq_t[5], wq.ap()[5 * 128:6 * 128, :])
            sync_dma(kv_t[2], kv_ap[2])
            bqa_t = pw.tile([HA, HPC], F32, tag="bqa")
            nc.sync.dma_start(bqa_t[:], bqa.ap())
            bka_t = pw.tile([HA, HPC], F32, tag="bka")
            nc.sync.dma_start(bka_t[:], bka.ap())
            for e in range(3, NE):
                sync_dma(kv_t[e], kv_ap[e])
            for e in range(NE):
                nc.gpsimd.dma_start(
                    xt[e][1][:], xT.ap()[e * 128:(e + 1) * 128, HQ:S])
            wo_t = []
            for h in range(HPC):
                t = pw.tile([HD, E], BF, tag=f"wo{h}", name=f"wo{h}")
                nc.sync.dma_start(t[:], wo.ap()[h])
                wo_t.append(t)

            # persistent per-head tiles
            qT = [[pw.tile([HA, HQ], F16, tag=f"qT{h}_{q}", name=f"qT{h}_{q}")
                   for q in range(QH)] for h in range(HPC)]
            kT = [[pw.tile([HA, HQ], F16, tag=f"kT{h}_{q}", name=f"kT{h}_{q}")
                   for q in range(QH)] for h in range(HPC)]
            aN = [pw.tile([HD, S], BF, tag=f"aN{h}", name=f"aN{h}")
                  for h in range(HPC)]
            vS = [None, None]

            # ---------- projection units (generators for interleaving) --
            def proj_unit(proj, col0, half, drain):
                """One [*,1024] projection: 12 matmuls + drain; yields
                after each matmul so attention work can interleave."""
                ps = pb.tile([128, HQ], F32, tag="pb", name="ps")
                for e in range(NE):
                    if proj == 0:
                        lhsT = wq_t[e][:, col0:col0 + 128]
                    else:
                        lhsT = kv_t[e][:, proj - 1, col0:col0 + 128]
                    for c2 in range(2):
                        nc.tensor.matmul(
                            ps[:, c2 * 512:(c2 + 1) * 512], lhsT,
                            xt[e][half][:, c2 * 512:(c2 + 1) * 512],
                            start=(e == 0), stop=(e == NE - 1))
                        yield
                drain(ps)
                yield

            def drain_q(h, half):
                def f(ps):
                    nc.scalar.activation(
                        qT[h][half][:], ps[0:HA, :], AF.Identity,
                        bias=bqa_t[:, h:h + 1])
                return f

            def drain_k(h, half):
                def f(ps):
                    nc.scalar.activation(
                        kT[h][half][:], ps[0:HA, :], AF.Identity,
                        bias=bka_t[:, h:h + 1])
                return f

            # vT / vS: V' with a built-in ones row (row HD) per head.
            # vS is split in two 2D tiles so each DMA transpose writes a
            # WHOLE tile -- transposes writing 3D-sliced APs are not
            # tracked correctly against later stationary reads.
            VR = 128  # full stationary width (HW wants M=128)
            vT = [pvt.tile([VR, S], BF, tag=f"vT{h}", name=f"vT{h}")
                  for h in range(HPC)]
            vS = [[pvt.tile([128, NT // 2, VR], BF, tag=f"vS{h}_{p}",
                            name=f"vS{h}_{p}") for p in range(2)]
                  for h in range(HPC)]

            def v_unit(h, half):
                if half == 0:
                    nc.gpsimd.memset(vT[h][HD:VR, :], 0.0)
                    nc.gpsimd.memset(vT[h][HD:HD + 1, :], 1.0)

                def dv(ps):
                    nc.vector.tensor_copy(
                        vT[h][0:HD, half * HQ:(half + 1) * HQ], ps[0:HD, :])
                yield from proj_unit(2, h * 128, half, dv)
                if half == 1:
                    # split transpose: key tiles 0..7 / 8..15
                    nc.sync.dma_start_transpose(vS[h][0][:], vT[h][:, 0:HQ])
                    nc.sync.dma_start_transpose(vS[h][1][:], vT[h][:, HQ:S])
                    yield

            def outproj_unit(qt):
                pf = pb.tile([128, HQ], F32, tag="pb", name="pf")
                if qt % 4 == 0:
                    # fence: moving-operand read of the attN columns this
                    # 512-chunk depends on (written by the normalize mul)
                    for h2 in range(HPC):
                        nc.tensor.matmul(
                            pf[:, 0:1], aN[h2][:, qt * 128:(qt + 1) * 128],
                            aN[h2][:, qt * 128:qt * 128 + 1],
                            start=True, stop=True)
                    yield
                for h2 in range(HPC):
                    lhsT = aN[h2][:, qt * 128:(qt + 1) * 128]
                    nc.tensor.matmul(
                        pf[:, 0:512], lhsT, wo_t[h2][:, 0:512],
                        start=(h2 == 0), stop=(h2 == HPC - 1))
                    yield
                    nc.tensor.matmul(
                        pf[:, 512:768], lhsT, wo_t[h2][:, 512:768],
                        start=(h2 == 0), stop=(h2 == HPC - 1))
                    yield
                ot = pout.tile([128, E], F16, tag="ot", name="ot")
                if qt % 2 == 0:
                    nc.vector.tensor_copy(ot[:], pf[:, 0:E])
                else:
                    nc.scalar.activation(ot[:], pf[:, 0:E], AF.Identity)
                if qt < 8:
                    eng = nc.sync if qt % 2 == 0 else nc.gpsimd
                    eng.dma_start(out.ap()[qt * 128:(qt + 1) * 128, :], ot[:])
                else:
                    # tail tiles: split rows across two issue queues so the
                    # last transfers drain in parallel (per-queue ~23 GB/s)
                    nc.sync.dma_start(
                        out.ap()[qt * 128:qt * 128 + 64, :], ot[0:64, :])
                    nc.gpsimd.dma_start(
                        out.ap()[qt * 128 + 64:(qt + 1) * 128, :], ot[64:128, :])
                yield

            # global ordered filler chain with labels
            def filler_chain():
                yield from v_unit(0, 0)
                yield from proj_unit(1, 0 * 128, 1, drain_k(0, 1))
                yield "k01"
                yield from v_unit(0, 1)
                yield "vS0"
                yield from proj_unit(0, 0 * 128, 1, drain_q(0, 1))
                yield "q01"
                yield from proj_unit(0, 1 * 128, 0, drain_q(1, 0))
                yield from proj_unit(1, 1 * 128, 0, drain_k(1, 0))
                yield "qk10"
                yield from v_unit(1, 0)
                yield from v_unit(1, 1)
                yield "vS1"
                yield from proj_unit(1, 1 * 128, 1, drain_k(1, 1))
                yield "k11"

            fill = filler_chain()
            done_labels = set()

            def feed(n):
                for _ in range(n):
                    for step in fill:
                        if isinstance(step, str):
                            done_labels.add(step)
                            continue
                        break
                    else:
                        return

            def feed_until(label):
                while label not in done_labels:
                    for step in fill:
                        if isinstance(step, str):
                            done_labels.add(step)
                            if step == label:
                                break
                        else:
                            break
                    else:
                        return

            # ---------- warm-up projections (not interleaved) -----------
            for _ in proj_unit(0, 0 * 128, 0, drain_q(0, 0)):
                pass
            for _ in proj_unit(1, 0 * 128, 0, drain_k(0, 0)):
                pass

            def fence_vs(att, h):
                # moving-operand reads carry proper waits; once these two
                # matmuls retire, the in-order PE queue is safe to load
                # vS tiles as stationary weights.
                for p in range(2):
                    nc.tensor.matmul(att[:, 0:1], xt[0][0][:, 0:128],
                                     vS[h][p][:, 0, 0:1],
                                     start=True, stop=True)

            # ---------- attention: 4 passes of 16 key tiles -------------
            def pv_mm(att, h, kt, pt):
                for c in range(2):
                    nc.tensor.matmul(
                        att[:, c * 512:(c + 1) * 512],
                        vS[h][kt // 8][:, kt % 8, :],
                        pt[:, c * 512:(c + 1) * 512],
                        start=(kt == 0), stop=(kt == NT - 1))

            def attention_pass(h, qh, fpk, interleave_pv, pv_prereq=None,
                               kt8_prereq=None, post=None, dummies=0,
                               dbg_cap=False):
                att = pa.tile([128, HQ], F32, tag="att", name="att")
                if interleave_pv and qh == 0:
                    fence_vs(att, h)
                pts = []
                for kt in range(NT):
                    if kt == 8 and kt8_prereq is not None:
                        # the second-half K tile must be EMITTED before any
                        # score matmul that reads it
                        feed_until(kt8_prereq)
                    sc = pb.tile([128, HQ], F32, tag="pb", name="sc")
                    kTt = kT[h][kt // 8]
                    k0 = (kt % 8) * 128
                    for _ in range(dummies):
                        # PE keep-alive: result discarded by the real
                        # start=True score matmul below
                        nc.tensor.matmul(sc[:, 0:512], kTt[:, k0:k0 + 128],
                                         qT[h][qh][:, 0:512],
                                         start=True, stop=True)
                    for c in range(2):
                        nc.tensor.matmul(
                            sc[:, c * 512:(c + 1) * 512],
                            kTt[:, k0:k0 + 128],
                            qT[h][qh][:, c * 512:(c + 1) * 512],
                            start=True, stop=True)
                    if interleave_pv and kt > 0:
                        pv_mm(att, h, kt - 1, pts[-1])
                    feed(fpk[kt] if isinstance(fpk, list) else fpk)
                    pt = ppr.tile([128, HQ], BF, tag="probsT", name="pt")
                    nc.scalar.activation(pt[:], sc[:], AF.Exp)
                    if dbg_cap and kt == 0 and _CACHE.get("debug"):
                        capp = pw.tile([128, HQ], F32, tag="dbgpt", name="capp")
                        nc.vector.tensor_copy(capp[:], pt[:])
                        nc.sync.dma_start(dbg["pt0"].ap(), capp[:])
                    pts.append(pt)
                if interleave_pv:
                    pv_mm(att, h, NT - 1, pts[-1])
                else:
                    if pv_prereq is not None:
                        feed_until(pv_prereq)
                    fence_vs(att, h)
                    for kt in range(NT):
                        pv_mm(att, h, kt, pts[kt])
                if dbg_cap and _CACHE.get("debug"):
                    cap = pw.tile([128, HQ], F32, tag="dbgcap", name="cap")
                    nc.vector.tensor_copy(cap[:], att[:])
                    nc.sync.dma_start(dbg["att00"].ap(), cap[:])
                # normalize in 512-col chunks.  The sums row must bounce
                # through SBUF (reciprocal reading PSUM directly returns
                # garbage on HW); the copy runs on the scalar engine,
                # which is idle once the pass's exps are done.  post(c)
                # emits dependent work (the matching output-projection
                # tiles) right after chunk c's multiply.
                for c in range(2):
                    cs = slice(c * 512, (c + 1) * 512)
                    sR = pno.tile([1, 512], F32, tag=f"sR{c}", name=f"sR{c}")
                    nc.vector.tensor_copy(sR[:], att[HD:HD + 1, cs])
                    rR = pno.tile([1, 512], F32, tag=f"rR{c}", name=f"rR{c}")
                    nc.vector.reciprocal_approx_fast(rR[:], sR[:])
                    rb = pno.tile([HD, 512], F32, tag=f"rb{c}", name=f"rb{c}")
                    nc.gpsimd.partition_broadcast(rb[:], rR[:])
                    nc.vector.tensor_mul(
                        aN[h][:, qh * HQ + c * 512:qh * HQ + (c + 1) * 512],
                        att[0:HD, cs], rb[:])
                    if post is not None:
                        post(c)

            def op_range(q0, q1):
                for qt in range(q0, q1):
                    for _ in outproj_unit(qt):
                        pass

            # pass (0,0): vS0 is emitted mid-pass by fillers, so PVs are
            # emitted as a block at the end (needs all 16 pt tiles live).
            attention_pass(0, 0, 3, interleave_pv=False, pv_prereq="vS0",
                           kt8_prereq="k01", dbg_cap=True)
            feed_until("q01")
            attention_pass(0, 1, 3, interleave_pv=True)
            feed_until("qk10")
            feed_until("vS1")
            attention_pass(1, 0, 3, interleave_pv=True, kt8_prereq="k11")
            feed(10000)  # flush any remaining projection fillers
            # q1's second-half Q projection runs here: it hides the head-1
            # qh0 normalize latency before the first output-projection block
            for _ in proj_unit(0, 1 * 128, 1, drain_q(1, 1)):
                pass
            pfd = pb.tile([128, HQ], F32, tag="pb", name="pfd")
            for _ in range(8):
                nc.tensor.matmul(pfd[:, 0:512], wo_t[0][:, 0:128],
                                 wo_t[0][:, 0:512], start=True, stop=True)
            op_range(0, 4)

            def post11(c):
                if c == 0:
                    # qt 4..7 are ready (normalized long ago); they hide
                    # the first normalize chunk's latency
                    op_range(4, 8)
                    op_range(8, 12)
                else:
                    op_range(12, 16)

            attention_pass(1, 1, 0, interleave_pv=True, post=post11,
                           dummies=1)

            if _CACHE.get("debug"):
                nc.sync.dma_start(dbg["qT00"].ap(), qT[0][0][:])
                nc.sync.dma_start(dbg["kT00"].ap(), kT[0][0][:])
                nc.sync.dma_start(dbg["kT01"].ap(), kT[0][1][:])
                nc.sync.dma_start(dbg["vS0"].ap()[:, 0:8, :], vS[0][0][:])
                nc.sync.dma_start(dbg["vS0"].ap()[:, 8:16, :], vS[0][1][:])
                nc.sync.dma_start(dbg["aN0"].ap(), aN[0][:])
                nc.sync.dma_start(dbg["aN1"].ap(), aN[1][:])

    nc.compile()
    return nc


def kernel(x, Wq, bq, Wk, bk, Wv, bv, Wo, bo):
    x = np.asarray(x, np.float32)
    Wq, bq = np.asarray(Wq, np.float32), np.asarray(bq, np.float32)
    Wk, bk = np.asarray(Wk, np.float32), np.asarray(bk, np.float32)
    Wv, bv = np.asarray(Wv, np.float32), np.asarray(bv, np.float32)
    Wo, bo = np.asarray(Wo, np.float32), np.asarray(bo, np.float32)

    if "nc" not in _CACHE:
        _CACHE["nc"] = _build()
    nc = _CACHE["nc"]

    bo_p = bo.astype(np.float64) + SCALING * (bv.astype(np.float64)
                                              @ Wo.astype(np.float64))

    in_maps = []
    for core in range(N_CORES):
        b = core // 4
        h0 = (core % 4) * HPC
        wq_a = np.zeros((E, HPC, 128), np.float32)
        wk_a = np.zeros((E, HPC, 128), np.float32)
        wv_s = np.zeros((E, HPC, 128), np.float32)
        wo_s = np.zeros((HPC, HD, E), np.float32)
        bqa = np.zeros((HA, HPC), np.float32)
        bka = np.zeros((HA, HPC), np.float32)
        for j in range(HPC):
            sl = slice((h0 + j) * HD, (h0 + j + 1) * HD)
            wq_a[:, j, 0:HD] = Wq[:, sl]
            wq_a[:, j, HD] = Wq[:, sl] @ bk[sl]
            # wq_a[:, j, HD+1] stays 0 (constant 1 comes from the bias)
            wk_a[:, j, 0:HD] = Wk[:, sl]
            # wk_a[:, j, HD] stays 0 (constant 1 via bias)
            wk_a[:, j, HD + 1] = Wk[:, sl] @ bq[sl]
            wv_s[:, j, 0:HD] = Wv[:, sl]
            wo_s[j] = SCALING * Wo[sl, :]
            bqa[HD, j] = float(bq[sl] @ bk[sl])
            bqa[HD + 1, j] = 1.0
            bka[HD, j] = 1.0
        wkv = np.stack([wk_a.reshape(E, HPC * 128),
                        wv_s.reshape(E, HPC * 128)], axis=1)
        in_maps.append({
            "xT": np.ascontiguousarray(x[b].T).astype(np.float16),
            "wq": wq_a.reshape(E, HPC * 128).astype(np.float16),
            "wkv": np.ascontiguousarray(wkv).reshape(
                E, 2 * HPC * 128).astype(np.float16),
            "wo": wo_s.astype(bf16),
            "bqa": bqa,
            "bka": bka,
        })

    res = bass_utils.run_bass_kernel_spmd(
        nc, in_maps, core_ids=list(range(N_CORES)))
    _CACHE["last_result"] = res

    parts = [res.results[i]["out"].astype(np.float64) for i in range(N_CORES)]
    full = np.stack([sum(parts[b * 4:(b + 1) * 4]) + bo_p for b in range(B)])
    return full.astype(np.float32)
